# revision 4
# baseline (speedup 1.0000x reference)
from contextlib import ExitStack

import numpy as np
import ml_dtypes

import concourse.bass as bass
import concourse.mybir as mybir
import concourse.tile as tile
from concourse import bacc
from concourse.bass_utils import run_bass_kernel_spmd

B, N, C, H, D = 4, 2048, 256, 4, 64
NCORES = 8
Q = 1024
SCALE = float(D) ** -0.5
FP32 = mybir.dt.float32
BF16 = mybir.dt.bfloat16
FP8E4 = mybir.dt.float8e4
FP8E5 = mybir.dt.float8e5
U8 = mybir.dt.uint8
AF = mybir.ActivationFunctionType
DR = mybir.MatmulPerfMode.DoubleRow
ALU = mybir.AluOpType

A5 = 4.0 * np.log2(np.e) * SCALE
B5 = 60.0

DVE_KCS = (3, 6, 9, 12, 15)

_CACHE = {}


def build_nc():
    nc = bacc.Bacc("TRN2", target_bir_lowering=False, debug=False,
                   num_devices=NCORES)

    t1T_d = nc.dram_tensor("t1T", [C, N], BF16, kind="ExternalInput")
    t2T_d = nc.dram_tensor("t2T", [C, Q], BF16, kind="ExternalInput")
    wq_d = nc.dram_tensor("wq", [C, C], BF16, kind="ExternalInput")
    wk_d = nc.dram_tensor("wk", [C, C], BF16, kind="ExternalInput")
    wv_d = nc.dram_tensor("wv", [C, C], BF16, kind="ExternalInput")
    wc_d = nc.dram_tensor("wc", [C, C], BF16, kind="ExternalInput")
    wph_d = nc.dram_tensor("wph", [4 * 65, C], BF16, kind="ExternalInput")
    out_d = nc.dram_tensor("out", [C, Q], FP32, kind="ExternalOutput")

    with tile.TileContext(nc) as tc, ExitStack() as ctx:
        const = ctx.enter_context(tc.tile_pool(name="const", bufs=1))
        acts = ctx.enter_context(tc.tile_pool(name="acts", bufs=1))

        w_sb = {}
        for name, dram in (("wk", wk_d), ("wq", wq_d), ("wv", wv_d),
                           ("wc", wc_d)):
            tiles = []
            for cc in range(2):
                t = const.tile([128, C], BF16, name=f"{name}{cc}",
                               tag=f"{name}{cc}")
                nc.sync.dma_start(out=t[:],
                                  in_=dram[cc * 128:(cc + 1) * 128, :])
                tiles.append(t)
            w_sb[name] = tiles
        wph = []
        for h in range(4):
            t = const.tile([65, C], BF16, name=f"wph{h}", tag=f"wph{h}")
            nc.sync.dma_start(out=t[:], in_=wph_d[h * 65:(h + 1) * 65, :])
            wph.append(t)

        t1T = [acts.tile([128, N], BF16, name=f"t1T{cc}", tag=f"t1T{cc}")
               for cc in range(2)]
        t2T = [acts.tile([128, Q], BF16, name=f"t2T{cc}", tag=f"t2T{cc}")
               for cc in range(2)]
        for nn in range(N // 512):
            for cc in range(2):
                nc.sync.dma_start(
                    out=t1T[cc][:, nn * 512:(nn + 1) * 512],
                    in_=t1T_d[cc * 128:(cc + 1) * 128, nn * 512:(nn + 1) * 512])
        for nn in range(Q // 512):
            for cc in range(2):
                nc.sync.dma_start(
                    out=t2T[cc][:, nn * 512:(nn + 1) * 512],
                    in_=t2T_d[cc * 128:(cc + 1) * 128, nn * 512:(nn + 1) * 512])

        kT = [acts.tile([128, N], BF16, name=f"kT{m}", tag=f"kT{m}")
              for m in range(2)]
        qT = [acts.tile([128, Q], BF16, name=f"qT{m}", tag=f"qT{m}")
              for m in range(2)]
        v_sb = acts.tile([128, 8 * 640], FP8E4, name="v", tag="v")
        for kcp in range(8):
            nc.gpsimd.memset(
                v_sb[:, kcp * 640:(kcp + 1) * 640]
                .rearrange("p (t h c) -> p t h c", t=2, h=4)[:, :, :, 0:1],
                1.0)
        xon = [acts.tile([65, Q], BF16, name=f"xon{h}", tag=f"xon{h}")
               for h in range(4)]

        pes_pool = ctx.enter_context(tc.tile_pool(name="pes", bufs=2))
        npool = ctx.enter_context(tc.tile_pool(name="norm", bufs=2))
        osb = ctx.enter_context(tc.tile_pool(name="osb", bufs=2))

        spool = ctx.enter_context(
            tc.tile_pool(name="spsum", bufs=2, space="PSUM"))
        attn_ctx = ExitStack()

        def emit_kT(m, blk):
            ps = spool.tile([128, 1024], FP32, name="s", tag="s")
            for j in range(2):
                for cc in range(2):
                    nc.tensor.matmul(
                        ps[:, j * 512:(j + 1) * 512],
                        lhsT=w_sb["wk"][cc][:, m * 128:(m + 1) * 128],
                        rhs=t1T[cc][:, blk * 1024 + j * 512:
                                    blk * 1024 + (j + 1) * 512],
                        start=(cc == 0), stop=(cc == 1))
            nc.vector.tensor_copy(
                kT[m][:, blk * 1024:(blk + 1) * 1024], ps[:])

        def emit_qT(m):
            ps = spool.tile([128, 1024], FP32, name="s", tag="s")
            for j in range(2):
                for cc in range(2):
                    nc.tensor.matmul(
                        ps[:, j * 512:(j + 1) * 512],
                        lhsT=w_sb["wq"][cc][:, m * 128:(m + 1) * 128],
                        rhs=t2T[cc][:, j * 512:(j + 1) * 512],
                        start=(cc == 0), stop=(cc == 1))
            nc.vector.tensor_copy(qT[m][:], ps[:])

        def emit_v(batch):
            ps = spool.tile([128, 1024], FP32, name="s", tag="s")
            for i in range(4):
                kc = batch * 4 + i
                for cc in range(2):
                    nc.tensor.matmul(
                        ps[:, i * 256:(i + 1) * 256],
                        lhsT=t1T[cc][:, kc * 128:(kc + 1) * 128],
                        rhs=w_sb["wv"][cc][:],
                        start=(cc == 0), stop=(cc == 1))
            src = ps[:].rearrange("p (k h c) -> p k h c", k=4, h=4)
            dst = v_sb[:, batch * 1280:(batch + 1) * 1280] \
                .rearrange("p (k h c) -> p k h c", k=4, h=4, c=80)[:, :, :, 1:65]
            nc.vector.tensor_copy(dst, src)

        def emit_s_exp(m, kc, j, pes_t):
            s = spool.tile([128, 1024], FP32, name="s", tag="s")
            for hh in range(2):
                nc.tensor.matmul(
                    s[:, hh * 512:(hh + 1) * 512],
                    lhsT=kT[m][hh * 64:(hh + 1) * 64,
                               kc * 128:(kc + 1) * 128],
                    rhs=qT[m][hh * 64:(hh + 1) * 64, j * 512:(j + 1) * 512],
                    start=True, stop=True)
            t = kc & 1
            if kc in DVE_KCS:
                dst = pes_t[:].bitcast(U8)[:, t * 1024:(t + 1) * 1024]
                nc.vector.tensor_scalar(dst, s[:], A5, B5, ALU.mult, ALU.add)
            else:
                nc.scalar.activation(pes_t[:, t * 1024:(t + 1) * 1024], s[:],
                                     AF.Exp, scale=SCALE)

        def emit_xo(m, kcp, xo_ps, pes_j):
            for j in range(2):
                rhs = pes_j[j][:].rearrange("p (t q) -> p t q", t=2)
                for hh in range(2):
                    h = 2 * m + hh
                    lhsT = v_sb[:, kcp * 640:(kcp + 1) * 640] \
                        .rearrange("p (t h c) -> p t h c", t=2, h=4)[:, :, h, 0:65]
                    nc.tensor.matmul(
                        xo_ps[hh][0:65, j * 512:(j + 1) * 512],
                        lhsT=lhsT,
                        rhs=rhs[:, :, hh * 512:(hh + 1) * 512],
                        start=(kcp == 0), stop=(kcp == 7),
                        perf_mode=DR)

        emit_kT(0, 0)
        emit_kT(0, 1)
        emit_qT(0)

        xopool = attn_ctx.enter_context(
            tc.tile_pool(name="xopsum", bufs=1, space="PSUM"))

        def attention(m, interleave):
            xo_ps = [xopool.tile([65, Q], FP32, name=f"xo{hh}",
                                 tag=f"xo{hh}") for hh in range(2)]
            for kcp in range(8):
                pes_j = [pes_pool.tile([128, 2048], FP8E5, name=f"pes{j}",
                                       tag=f"pes{j}") for j in range(2)]
                for t in range(2):
                    for j in range(2):
                        emit_s_exp(m, 2 * kcp + t, j, pes_j[j])
                emit_xo(m, kcp, xo_ps, pes_j)
                for fn in interleave.pop(kcp, []):
                    fn()
            for hh in range(2):
                recip = npool.tile([1, Q], FP32, name=f"recip{hh}",
                                   tag=f"recip{hh}")
                nc.vector.reciprocal_approx_fast(recip[:], xo_ps[hh][0:1, :])
                bc = npool.tile([65, Q], FP32, name=f"bc{hh}", tag=f"bc{hh}")
                nc.gpsimd.partition_broadcast(bc[:], recip[:])
                nc.vector.tensor_mul(xon[2 * m + hh][:], xo_ps[hh][0:65, :],
                                     bc[:])

        emit_v(0)
        emit_v(1)
        inter0 = {
            1: [lambda: emit_v(2)],
            2: [lambda: emit_v(3)],
            3: [lambda: emit_kT(1, 0)],
            4: [lambda: emit_kT(1, 1)],
            5: [lambda: emit_qT(1)],
        }
        attention(0, inter0)

        ot = []
        partialA = []

        def emit_projA():
            for ch in range(2):
                o = spool.tile([128, 1024], FP32, name="s", tag="s")
                ot.append(o)
                for j in range(2):
                    for cc in range(2):
                        nc.tensor.matmul(
                            o[:, j * 512:(j + 1) * 512],
                            lhsT=w_sb["wc"][cc][:, ch * 128:(ch + 1) * 128],
                            rhs=t2T[cc][:, j * 512:(j + 1) * 512],
                            start=(cc == 0), stop=False)
                    for h in range(2):
                        nc.tensor.matmul(
                            o[:, j * 512:(j + 1) * 512],
                            lhsT=wph[h][:, ch * 128:(ch + 1) * 128],
                            rhs=xon[h][:, j * 512:(j + 1) * 512],
                            start=False, stop=False)

        attention(1, {})
        emit_projA()

        for ch in range(2):
            o = ot[ch]
            for j in range(2):
                for h in range(2, 4):
                    nc.tensor.matmul(
                        o[:, j * 512:(j + 1) * 512],
                        lhsT=wph[h][:, ch * 128:(ch + 1) * 128],
                        rhs=xon[h][:, j * 512:(j + 1) * 512],
                        start=False, stop=(h == 3))
            o_sb = osb.tile([128, 1024], FP32, name="o", tag="o")
            nc.vector.tensor_copy(o_sb[:], o[:])
            nc.sync.dma_start(out=out_d[ch * 128:(ch + 1) * 128, :],
                              in_=o_sb[:])

        attn_ctx.close()

    nc.finalize()
    return nc


def _get_nc():
    if "nc" not in _CACHE:
        _CACHE["nc"] = build_nc()
    return _CACHE["nc"]


def make_in_maps(t2_grad, t1, Wq, Wkv, Wproj, bproj):
    bf16 = ml_dtypes.bfloat16
    t2 = np.asarray(t2_grad, dtype=np.float32)
    t1 = np.asarray(t1, dtype=np.float32)
    wq = np.ascontiguousarray(Wq, dtype=np.float32)
    wk = np.ascontiguousarray(Wkv[:, :C]).astype(bf16)
    wv = np.ascontiguousarray(Wkv[:, C:]).astype(bf16)
    wp64 = np.asarray(Wproj, dtype=np.float64)
    wcomb = ((np.eye(C) + np.asarray(Wq, np.float64)) @ wp64).astype(bf16)
    wph = np.zeros((4 * 65, C), dtype=np.float32)
    for h in range(4):
        wph[h * 65 + 1:(h + 1) * 65] = np.asarray(Wproj[h * 64:(h + 1) * 64],
                                                  np.float32)
    wph[0] = np.asarray(bproj, np.float32)
    wph = wph.astype(bf16)
    wq_b = wq.astype(bf16)

    in_maps = []
    for c in range(NCORES):
        b, qh = c // 2, c % 2
        in_maps.append({
            "t1T": np.ascontiguousarray(t1[b].T).astype(bf16),
            "t2T": np.ascontiguousarray(t2[b].T[:, qh * Q:(qh + 1) * Q]).astype(bf16),
            "wq": wq_b, "wk": wk, "wv": wv, "wc": wcomb, "wph": wph,
        })
    return in_maps


def kernel(t2_grad, t1, Wq, Wkv, Wproj, bproj, gamma, _trace=False,
           _use_fp32r=None):
    gamma = np.asarray(gamma)
    if float(np.abs(gamma).max()) != 0.0:
        return _host_reference(t2_grad, t1, Wq, Wkv, Wproj, bproj, gamma)

    nc = _get_nc()
    in_maps = make_in_maps(t2_grad, t1, Wq, Wkv, Wproj, bproj)
    res = run_bass_kernel_spmd(nc, in_maps, list(range(NCORES)), trace=_trace)
    out = np.empty((B, N, C), dtype=np.float32)
    for c in range(NCORES):
        b, qh = c // 2, c % 2
        out[b, qh * Q:(qh + 1) * Q, :] = res.results[c]["out"].T
    if _trace:
        _CACHE["last_result"] = res
    return out


def _host_reference(t2_grad, t1, Wq, Wkv, Wproj, bproj, gamma):
    t2 = np.asarray(t2_grad, dtype=np.float64)
    t1 = np.asarray(t1, dtype=np.float64)
    Wq = np.asarray(Wq, dtype=np.float64)
    Wkv = np.asarray(Wkv, dtype=np.float64)
    Wproj = np.asarray(Wproj, dtype=np.float64)
    bproj = np.asarray(bproj, dtype=np.float64)
    g = float(np.asarray(gamma).reshape(-1)[0])
    q = (t2 @ Wq).reshape(B, N, H, D).transpose(0, 2, 1, 3)
    kv = (t1 @ Wkv).reshape(B, N, 2, H, D).transpose(2, 0, 3, 1, 4)
    k, v = kv[0], kv[1]
    s = np.einsum('bhnd,bhmd->bhnm', q, k) * SCALE
    s = s - s.max(axis=-1, keepdims=True)
    p = np.exp(s)
    p /= p.sum(axis=-1, keepdims=True)
    x = np.einsum('bhnm,bhmd->bhnd', p, v)
    xp = x.transpose(0, 3, 1, 2).reshape(B, D, H * N)
    energy = xp @ xp.transpose(0, 2, 1)
    energy = energy - energy.max(axis=-1, keepdims=True)
    att = np.exp(energy)
    att /= att.sum(axis=-1, keepdims=True)
    lam_out = (att @ xp).reshape(B, D, H, N)
    lam_out = g * lam_out + xp.reshape(B, D, H, N)
    x = lam_out.transpose(0, 2, 3, 1)
    xo = x.transpose(0, 2, 1, 3).reshape(B, N, C) \
        + q.transpose(0, 2, 1, 3).reshape(B, N, C)
    return ((t2 + xo) @ Wproj + bproj).astype(np.float32)


# revision 7
# speedup vs baseline: 1.0629x; 1.0629x over previous
from contextlib import ExitStack

import numpy as np
import ml_dtypes

import concourse.bass as bass
import concourse.mybir as mybir
import concourse.tile as tile
from concourse import bacc
from concourse.bass_utils import run_bass_kernel_spmd

B, N, C, H, D = 4, 2048, 256, 4, 64
NCORES = 8
Q = 1024
SCALE = float(D) ** -0.5
FP32 = mybir.dt.float32
BF16 = mybir.dt.bfloat16
FP8E4 = mybir.dt.float8e4
FP8E5 = mybir.dt.float8e5
U8 = mybir.dt.uint8
AF = mybir.ActivationFunctionType
DR = mybir.MatmulPerfMode.DoubleRow
ALU = mybir.AluOpType

A5 = 4.0 * np.log2(np.e) * SCALE
B5 = 60.0

DVE_KCPS = ({4, 6}, {3, 5, 7})

_CACHE = {}


def build_nc():
    nc = bacc.Bacc("TRN2", target_bir_lowering=False, debug=False,
                   num_devices=NCORES)

    t1T_d = nc.dram_tensor("t1T", [C, N], BF16, kind="ExternalInput")
    t2T_d = nc.dram_tensor("t2T", [C, Q], BF16, kind="ExternalInput")
    wq_d = nc.dram_tensor("wq", [C, C], BF16, kind="ExternalInput")
    wk_d = nc.dram_tensor("wk", [C, C], BF16, kind="ExternalInput")
    wv_d = nc.dram_tensor("wv", [C, C], BF16, kind="ExternalInput")
    wc_d = nc.dram_tensor("wc", [C, C], BF16, kind="ExternalInput")
    wph_d = nc.dram_tensor("wph", [4 * 65, C], BF16, kind="ExternalInput")
    out_d = nc.dram_tensor("out", [C, Q], FP32, kind="ExternalOutput")

    with tile.TileContext(nc) as tc, ExitStack() as ctx:
        const = ctx.enter_context(tc.tile_pool(name="const", bufs=1))
        acts = ctx.enter_context(tc.tile_pool(name="acts", bufs=1))

        w_sb = {}
        for name, dram in (("wk", wk_d), ("wq", wq_d), ("wv", wv_d),
                           ("wc", wc_d)):
            tiles = []
            for cc in range(2):
                t = const.tile([128, C], BF16, name=f"{name}{cc}",
                               tag=f"{name}{cc}")
                nc.sync.dma_start(out=t[:],
                                  in_=dram[cc * 128:(cc + 1) * 128, :])
                tiles.append(t)
            w_sb[name] = tiles
        wph = []
        for h in range(4):
            t = const.tile([65, C], BF16, name=f"wph{h}", tag=f"wph{h}")
            nc.sync.dma_start(out=t[:], in_=wph_d[h * 65:(h + 1) * 65, :])
            wph.append(t)

        t1T = [acts.tile([128, N], BF16, name=f"t1T{cc}", tag=f"t1T{cc}")
               for cc in range(2)]
        t2T = [acts.tile([128, Q], BF16, name=f"t2T{cc}", tag=f"t2T{cc}")
               for cc in range(2)]
        for nn in range(N // 512):
            for cc in range(2):
                nc.sync.dma_start(
                    out=t1T[cc][:, nn * 512:(nn + 1) * 512],
                    in_=t1T_d[cc * 128:(cc + 1) * 128, nn * 512:(nn + 1) * 512])
        for nn in range(Q // 512):
            for cc in range(2):
                nc.sync.dma_start(
                    out=t2T[cc][:, nn * 512:(nn + 1) * 512],
                    in_=t2T_d[cc * 128:(cc + 1) * 128, nn * 512:(nn + 1) * 512])

        kT = [acts.tile([128, N], BF16, name=f"kT{m}", tag=f"kT{m}")
              for m in range(2)]
        qT = [acts.tile([128, Q], BF16, name=f"qT{m}", tag=f"qT{m}")
              for m in range(2)]
        v_sb = acts.tile([128, 8 * 640], FP8E4, name="v", tag="v")
        for kcp in range(8):
            nc.gpsimd.memset(
                v_sb[:, kcp * 640:(kcp + 1) * 640]
                .rearrange("p (t h c) -> p t h c", t=2, h=4)[:, :, :, 0:1],
                1.0)
        xon = [acts.tile([65, Q], BF16, name=f"xon{h}", tag=f"xon{h}")
               for h in range(4)]

        pes_pool = ctx.enter_context(tc.tile_pool(name="pes", bufs=5))
        npool = ctx.enter_context(tc.tile_pool(name="norm", bufs=2))
        osb = ctx.enter_context(tc.tile_pool(name="osb", bufs=2))

        spool = ctx.enter_context(
            tc.tile_pool(name="spsum", bufs=2, space="PSUM"))
        attn_ctx = ExitStack()
        phase1_ctx = ExitStack()
        ppool = phase1_ctx.enter_context(
            tc.tile_pool(name="ppsum", bufs=2, space="PSUM"))

        warm = const.tile([1, 16], FP32, name="warm", tag="warm")
        nc.gpsimd.memset(warm[:], 0.0)
        nc.scalar.activation(warm[:], warm[:], AF.Exp)

        def emit_kT(m, blk):
            ps = ppool.tile([128, 1024], FP32, name="p", tag="p")
            for j in range(2):
                for cc in range(2):
                    nc.tensor.matmul(
                        ps[:, j * 512:(j + 1) * 512],
                        lhsT=w_sb["wk"][cc][:, m * 128:(m + 1) * 128],
                        rhs=t1T[cc][:, blk * 1024 + j * 512:
                                    blk * 1024 + (j + 1) * 512],
                        start=(cc == 0), stop=(cc == 1))
            nc.vector.tensor_copy(
                kT[m][:, blk * 1024:(blk + 1) * 1024], ps[:])

        def emit_qT(m):
            ps = ppool.tile([128, 1024], FP32, name="p", tag="p")
            for j in range(2):
                for cc in range(2):
                    nc.tensor.matmul(
                        ps[:, j * 512:(j + 1) * 512],
                        lhsT=w_sb["wq"][cc][:, m * 128:(m + 1) * 128],
                        rhs=t2T[cc][:, j * 512:(j + 1) * 512],
                        start=(cc == 0), stop=(cc == 1))
            nc.vector.tensor_copy(qT[m][:], ps[:])

        def emit_v(batch):
            ps = ppool.tile([128, 1024], FP32, name="p", tag="p")
            for i in range(4):
                kc = batch * 4 + i
                for cc in range(2):
                    nc.tensor.matmul(
                        ps[:, i * 256:(i + 1) * 256],
                        lhsT=t1T[cc][:, kc * 128:(kc + 1) * 128],
                        rhs=w_sb["wv"][cc][:],
                        start=(cc == 0), stop=(cc == 1))
            src = ps[:].rearrange("p (k h c) -> p k h c", k=4, h=4)
            dst = v_sb[:, batch * 1280:(batch + 1) * 1280] \
                .rearrange("p (k h c) -> p k h c", k=4, h=4, c=80)[:, :, :, 1:65]
            nc.vector.tensor_copy(dst, src)

        def emit_s_exp(m, kc, j, pes_t):
            s = spool.tile([128, 1024], FP32, name="s", tag="s")
            for hh in range(2):
                nc.tensor.matmul(
                    s[:, hh * 512:(hh + 1) * 512],
                    lhsT=kT[m][hh * 64:(hh + 1) * 64,
                               kc * 128:(kc + 1) * 128],
                    rhs=qT[m][hh * 64:(hh + 1) * 64, j * 512:(j + 1) * 512],
                    start=True, stop=True)
            t = kc & 1
            if (kc // 2) in DVE_KCPS[m]:
                dst = pes_t[:].bitcast(U8)[:, t * 1024:(t + 1) * 1024]
                nc.vector.tensor_scalar(dst, s[:], A5, B5, ALU.mult, ALU.add)
            else:
                nc.scalar.activation(pes_t[:, t * 1024:(t + 1) * 1024], s[:],
                                     AF.Exp, scale=SCALE)

        def emit_xo(m, kcp, xo_ps, pes_j):
            for j in range(2):
                rhs = pes_j[j][:].rearrange("p (t q) -> p t q", t=2)
                for hh in range(2):
                    h = 2 * m + hh
                    lhsT = v_sb[:, kcp * 640:(kcp + 1) * 640] \
                        .rearrange("p (t h c) -> p t h c", t=2, h=4)[:, :, h, 0:65]
                    nc.tensor.matmul(
                        xo_ps[hh][0:65, j * 512:(j + 1) * 512],
                        lhsT=lhsT,
                        rhs=rhs[:, :, hh * 512:(hh + 1) * 512],
                        start=(kcp == 0), stop=(kcp == 7),
                        perf_mode=DR)

        emit_kT(0, 0)
        emit_kT(0, 1)
        emit_qT(0)

        def norm(m, xo_ps):
            for hh in range(2):
                recip = npool.tile([1, Q], FP32, name=f"recip{hh}",
                                   tag=f"recip{hh}")
                nc.vector.reciprocal_approx_fast(recip[:], xo_ps[hh][0:1, :])
                bc = npool.tile([65, Q], FP32, name=f"bc{hh}", tag=f"bc{hh}")
                nc.gpsimd.partition_broadcast(bc[:], recip[:])
                nc.vector.tensor_mul(xon[2 * m + hh][:], xo_ps[hh][0:65, :],
                                     bc[:])

        pes_m0 = {}
        phase1_work = [lambda: emit_v(0), lambda: emit_v(1),
                       lambda: emit_v(2), lambda: emit_v(3),
                       lambda: emit_kT(1, 0), lambda: emit_kT(1, 1),
                       lambda: emit_qT(1)]
        N_EARLY = 6
        for kc in range(N_EARLY):
            kcp, t = kc // 2, kc & 1
            if t == 0:
                pes_m0[kcp] = [pes_pool.tile([128, 2048], FP8E5,
                                             name=f"pes{j}", tag=f"pes{j}")
                               for j in range(2)]
            for j in range(2):
                emit_s_exp(0, kc, j, pes_m0[kcp][j])
            if phase1_work:
                phase1_work.pop(0)()
            if phase1_work:
                phase1_work.pop(0)()
        phase1_ctx.close()

        xopool = attn_ctx.enter_context(
            tc.tile_pool(name="xopsum", bufs=1, space="PSUM"))
        xo_ps0 = [xopool.tile([65, Q], FP32, name=f"xo{hh}", tag=f"xo{hh}")
                  for hh in range(2)]
        for kcp in range(N_EARLY // 2):
            emit_xo(0, kcp, xo_ps0, pes_m0.pop(kcp))
        for kc in range(N_EARLY, 16):
            kcp, t = kc // 2, kc & 1
            if t == 0:
                pes_m0[kcp] = [pes_pool.tile([128, 2048], FP8E5,
                                             name=f"pes{j}", tag=f"pes{j}")
                               for j in range(2)]
            for j in range(2):
                emit_s_exp(0, kc, j, pes_m0[kcp][j])
            if t == 1:
                emit_xo(0, kcp, xo_ps0, pes_m0.pop(kcp))
        norm(0, xo_ps0)

        def attention(m, interleave):
            xo_ps = [xopool.tile([65, Q], FP32, name=f"xo{hh}",
                                 tag=f"xo{hh}") for hh in range(2)]
            for kcp in range(8):
                pes_j = [pes_pool.tile([128, 2048], FP8E5, name=f"pes{j}",
                                       tag=f"pes{j}") for j in range(2)]
                for t in range(2):
                    for j in range(2):
                        emit_s_exp(m, 2 * kcp + t, j, pes_j[j])
                emit_xo(m, kcp, xo_ps, pes_j)
                for fn in interleave.pop(kcp, []):
                    fn()
            norm(m, xo_ps)

        ot = []
        partialA = []

        def emit_projA():
            for ch in range(2):
                o = spool.tile([128, 1024], FP32, name="s", tag="s")
                ot.append(o)
                for j in range(2):
                    for cc in range(2):
                        nc.tensor.matmul(
                            o[:, j * 512:(j + 1) * 512],
                            lhsT=w_sb["wc"][cc][:, ch * 128:(ch + 1) * 128],
                            rhs=t2T[cc][:, j * 512:(j + 1) * 512],
                            start=(cc == 0), stop=False)
                    for h in range(2):
                        nc.tensor.matmul(
                            o[:, j * 512:(j + 1) * 512],
                            lhsT=wph[h][:, ch * 128:(ch + 1) * 128],
                            rhs=xon[h][:, j * 512:(j + 1) * 512],
                            start=False, stop=False)

        attention(1, {})
        emit_projA()

        for ch in range(2):
            o = ot[ch]
            for j in range(2):
                for h in range(2, 4):
                    nc.tensor.matmul(
                        o[:, j * 512:(j + 1) * 512],
                        lhsT=wph[h][:, ch * 128:(ch + 1) * 128],
                        rhs=xon[h][:, j * 512:(j + 1) * 512],
                        start=False, stop=(h == 3))
            o_sb = osb.tile([128, 1024], FP32, name="o", tag="o")
            nc.vector.tensor_copy(o_sb[:], o[:])
            nc.sync.dma_start(out=out_d[ch * 128:(ch + 1) * 128, :],
                              in_=o_sb[:])

        attn_ctx.close()

    nc.finalize()
    return nc


def _get_nc():
    if "nc" not in _CACHE:
        _CACHE["nc"] = build_nc()
    return _CACHE["nc"]


def make_in_maps(t2_grad, t1, Wq, Wkv, Wproj, bproj):
    bf16 = ml_dtypes.bfloat16
    t2 = np.asarray(t2_grad, dtype=np.float32)
    t1 = np.asarray(t1, dtype=np.float32)
    wq = np.ascontiguousarray(Wq, dtype=np.float32)
    wk = np.ascontiguousarray(Wkv[:, :C]).astype(bf16)
    wv = np.ascontiguousarray(Wkv[:, C:]).astype(bf16)
    wp64 = np.asarray(Wproj, dtype=np.float64)
    wcomb = ((np.eye(C) + np.asarray(Wq, np.float64)) @ wp64).astype(bf16)
    wph = np.zeros((4 * 65, C), dtype=np.float32)
    for h in range(4):
        wph[h * 65 + 1:(h + 1) * 65] = np.asarray(Wproj[h * 64:(h + 1) * 64],
                                                  np.float32)
    wph[0] = np.asarray(bproj, np.float32)
    wph = wph.astype(bf16)
    wq_b = wq.astype(bf16)

    in_maps = []
    for c in range(NCORES):
        b, qh = c // 2, c % 2
        in_maps.append({
            "t1T": np.ascontiguousarray(t1[b].T).astype(bf16),
            "t2T": np.ascontiguousarray(t2[b].T[:, qh * Q:(qh + 1) * Q]).astype(bf16),
            "wq": wq_b, "wk": wk, "wv": wv, "wc": wcomb, "wph": wph,
        })
    return in_maps


def kernel(t2_grad, t1, Wq, Wkv, Wproj, bproj, gamma, _trace=False,
           _use_fp32r=None):
    gamma = np.asarray(gamma)
    if float(np.abs(gamma).max()) != 0.0:
        return _host_reference(t2_grad, t1, Wq, Wkv, Wproj, bproj, gamma)

    nc = _get_nc()
    in_maps = make_in_maps(t2_grad, t1, Wq, Wkv, Wproj, bproj)
    res = run_bass_kernel_spmd(nc, in_maps, list(range(NCORES)), trace=_trace)
    out = np.empty((B, N, C), dtype=np.float32)
    for c in range(NCORES):
        b, qh = c // 2, c % 2
        out[b, qh * Q:(qh + 1) * Q, :] = res.results[c]["out"].T
    if _trace:
        _CACHE["last_result"] = res
    return out


def _host_reference(t2_grad, t1, Wq, Wkv, Wproj, bproj, gamma):
    t2 = np.asarray(t2_grad, dtype=np.float64)
    t1 = np.asarray(t1, dtype=np.float64)
    Wq = np.asarray(Wq, dtype=np.float64)
    Wkv = np.asarray(Wkv, dtype=np.float64)
    Wproj = np.asarray(Wproj, dtype=np.float64)
    bproj = np.asarray(bproj, dtype=np.float64)
    g = float(np.asarray(gamma).reshape(-1)[0])
    q = (t2 @ Wq).reshape(B, N, H, D).transpose(0, 2, 1, 3)
    kv = (t1 @ Wkv).reshape(B, N, 2, H, D).transpose(2, 0, 3, 1, 4)
    k, v = kv[0], kv[1]
    s = np.einsum('bhnd,bhmd->bhnm', q, k) * SCALE
    s = s - s.max(axis=-1, keepdims=True)
    p = np.exp(s)
    p /= p.sum(axis=-1, keepdims=True)
    x = np.einsum('bhnm,bhmd->bhnd', p, v)
    xp = x.transpose(0, 3, 1, 2).reshape(B, D, H * N)
    energy = xp @ xp.transpose(0, 2, 1)
    energy = energy - energy.max(axis=-1, keepdims=True)
    att = np.exp(energy)
    att /= att.sum(axis=-1, keepdims=True)
    lam_out = (att @ xp).reshape(B, D, H, N)
    lam_out = g * lam_out + xp.reshape(B, D, H, N)
    x = lam_out.transpose(0, 2, 3, 1)
    xo = x.transpose(0, 2, 1, 3).reshape(B, N, C) \
        + q.transpose(0, 2, 1, 3).reshape(B, N, C)
    return ((t2 + xo) @ Wproj + bproj).astype(np.float32)


# revision 11
# speedup vs baseline: 1.1177x; 1.0516x over previous
from contextlib import ExitStack

import numpy as np
import ml_dtypes

import concourse.bass as bass
import concourse.mybir as mybir
import concourse.tile as tile
from concourse import bacc
from concourse.bass_utils import run_bass_kernel_spmd

B, N, C, H, D = 4, 2048, 256, 4, 64
NCORES = 8
Q = 1024
SCALE = float(D) ** -0.5
FP32 = mybir.dt.float32
BF16 = mybir.dt.bfloat16
FP8E4 = mybir.dt.float8e4
FP8E5 = mybir.dt.float8e5
U8 = mybir.dt.uint8
AF = mybir.ActivationFunctionType
DR = mybir.MatmulPerfMode.DoubleRow
ALU = mybir.AluOpType

A5 = 4.0 * np.log2(np.e) * SCALE
B5 = 60.0

DVE_GKCPS = {3, 6, 9, 12, 15}

_CACHE = {}


def build_nc():
    nc = bacc.Bacc("TRN2", target_bir_lowering=False, debug=False,
                   num_devices=NCORES)

    t1T_d = nc.dram_tensor("t1T", [C, N], BF16, kind="ExternalInput")
    t2T_d = nc.dram_tensor("t2T", [C, Q], BF16, kind="ExternalInput")
    wq_d = nc.dram_tensor("wq", [C, C], BF16, kind="ExternalInput")
    wk_d = nc.dram_tensor("wk", [C, C], BF16, kind="ExternalInput")
    wv_d = nc.dram_tensor("wv", [C, C], BF16, kind="ExternalInput")
    wc_d = nc.dram_tensor("wc", [C, C], BF16, kind="ExternalInput")
    wph_d = nc.dram_tensor("wph", [4 * 65, C], BF16, kind="ExternalInput")
    out_d = nc.dram_tensor("out", [C, Q], BF16, kind="ExternalOutput")

    with tile.TileContext(nc) as tc, ExitStack() as ctx:
        const = ctx.enter_context(tc.tile_pool(name="const", bufs=1))
        acts = ctx.enter_context(tc.tile_pool(name="acts", bufs=1))

        w_sb = {}
        for name, dram in (("wk", wk_d), ("wq", wq_d), ("wv", wv_d),
                           ("wc", wc_d)):
            tiles = []
            for cc in range(2):
                t = const.tile([128, C], BF16, name=f"{name}{cc}",
                               tag=f"{name}{cc}")
                nc.gpsimd.dma_start(out=t[:],
                                    in_=dram[cc * 128:(cc + 1) * 128, :])
                tiles.append(t)
            w_sb[name] = tiles
        wph = []
        for h in range(4):
            t = const.tile([65, C], BF16, name=f"wph{h}", tag=f"wph{h}")
            nc.gpsimd.dma_start(out=t[:], in_=wph_d[h * 65:(h + 1) * 65, :])
            wph.append(t)

        t1T = [acts.tile([128, N], BF16, name=f"t1T{cc}", tag=f"t1T{cc}")
               for cc in range(2)]
        t2T = [acts.tile([128, Q], BF16, name=f"t2T{cc}", tag=f"t2T{cc}")
               for cc in range(2)]
        for nn in range(2):
            for cc in range(2):
                nc.sync.dma_start(
                    out=t1T[cc][:, nn * 1024:(nn + 1) * 1024],
                    in_=t1T_d[cc * 128:(cc + 1) * 128,
                              nn * 1024:(nn + 1) * 1024])
        for cc in range(2):
            nc.scalar.dma_start(out=t2T[cc][:], in_=t2T_d[cc * 128:(cc + 1) * 128, :])

        kT = [acts.tile([128, N], BF16, name=f"kT{m}", tag=f"kT{m}")
              for m in range(2)]
        qT = [acts.tile([128, Q], BF16, name=f"qT{m}", tag=f"qT{m}")
              for m in range(2)]
        v_sb = acts.tile([128, 8 * 640], FP8E4, name="v", tag="v")
        for kcp in range(8):
            nc.gpsimd.memset(
                v_sb[:, kcp * 640:(kcp + 1) * 640]
                .rearrange("p (t h c) -> p t h c", t=2, h=4)[:, :, :, 0:1],
                1.0)
        xon = [acts.tile([65, Q], BF16, name=f"xon{h}", tag=f"xon{h}")
               for h in range(4)]

        pes_pool = ctx.enter_context(tc.tile_pool(name="pes", bufs=5))
        npool = ctx.enter_context(tc.tile_pool(name="norm", bufs=2))
        osb = ctx.enter_context(tc.tile_pool(name="osb", bufs=2))

        spool = ctx.enter_context(
            tc.tile_pool(name="spsum", bufs=2, space="PSUM"))
        attn_ctx = ExitStack()
        phase1_ctx = ExitStack()
        ppool = phase1_ctx.enter_context(
            tc.tile_pool(name="ppsum", bufs=2, space="PSUM"))

        warm = const.tile([1, 16], FP32, name="warm", tag="warm")
        nc.gpsimd.memset(warm[:], 0.0)
        nc.scalar.activation(warm[:], warm[:], AF.Exp)

        def emit_kT(m, blk):
            ps = ppool.tile([128, 1024], FP32, name="p", tag="p")
            for j in range(2):
                for cc in range(2):
                    nc.tensor.matmul(
                        ps[:, j * 512:(j + 1) * 512],
                        lhsT=w_sb["wk"][cc][:, m * 128:(m + 1) * 128],
                        rhs=t1T[cc][:, blk * 1024 + j * 512:
                                    blk * 1024 + (j + 1) * 512],
                        start=(cc == 0), stop=(cc == 1))
            nc.vector.tensor_copy(
                kT[m][:, blk * 1024:(blk + 1) * 1024], ps[:])

        def emit_qT(m):
            ps = ppool.tile([128, 1024], FP32, name="p", tag="p")
            for j in range(2):
                for cc in range(2):
                    nc.tensor.matmul(
                        ps[:, j * 512:(j + 1) * 512],
                        lhsT=w_sb["wq"][cc][:, m * 128:(m + 1) * 128],
                        rhs=t2T[cc][:, j * 512:(j + 1) * 512],
                        start=(cc == 0), stop=(cc == 1))
            nc.vector.tensor_copy(qT[m][:], ps[:])

        def emit_v(batch):
            ps = ppool.tile([128, 1024], FP32, name="p", tag="p")
            for i in range(4):
                kc = batch * 4 + i
                for cc in range(2):
                    nc.tensor.matmul(
                        ps[:, i * 256:(i + 1) * 256],
                        lhsT=t1T[cc][:, kc * 128:(kc + 1) * 128],
                        rhs=w_sb["wv"][cc][:],
                        start=(cc == 0), stop=(cc == 1))
            src = ps[:].rearrange("p (k h c) -> p k h c", k=4, h=4)
            dst = v_sb[:, batch * 1280:(batch + 1) * 1280] \
                .rearrange("p (k h c) -> p k h c", k=4, h=4, c=80)[:, :, :, 1:65]
            nc.vector.tensor_copy(dst, src)

        def emit_s_exp(m, kc, j, pes_t):
            s = spool.tile([128, 1024], FP32, name="s", tag="s")
            for hh in range(2):
                nc.tensor.matmul(
                    s[:, hh * 512:(hh + 1) * 512],
                    lhsT=kT[m][hh * 64:(hh + 1) * 64,
                               kc * 128:(kc + 1) * 128],
                    rhs=qT[m][hh * 64:(hh + 1) * 64, j * 512:(j + 1) * 512],
                    start=True, stop=True)
            t = kc & 1
            if (m * 8 + kc // 2) in DVE_GKCPS:
                dst = pes_t[:].bitcast(U8)[:, t * 1024:(t + 1) * 1024]
                nc.vector.tensor_scalar(dst, s[:], A5, B5, ALU.mult, ALU.add)
            else:
                nc.scalar.activation(pes_t[:, t * 1024:(t + 1) * 1024], s[:],
                                     AF.Exp, scale=SCALE)

        def emit_xo(m, kcp, xo_ps, pes_j):
            for j in range(2):
                rhs = pes_j[j][:].rearrange("p (t q) -> p t q", t=2)
                for hh in range(2):
                    h = 2 * m + hh
                    lhsT = v_sb[:, kcp * 640:(kcp + 1) * 640] \
                        .rearrange("p (t h c) -> p t h c", t=2, h=4)[:, :, h, 0:65]
                    nc.tensor.matmul(
                        xo_ps[hh][0:65, j * 512:(j + 1) * 512],
                        lhsT=lhsT,
                        rhs=rhs[:, :, hh * 512:(hh + 1) * 512],
                        start=(kcp == 0), stop=(kcp == 7),
                        perf_mode=DR)

        emit_kT(0, 0)
        emit_kT(0, 1)
        emit_qT(0)

        def norm(m, xo_ps):
            for hh in range(2):
                recip = npool.tile([1, Q], FP32, name=f"recip{hh}",
                                   tag=f"recip{hh}")
                nc.vector.reciprocal_approx_fast(recip[:], xo_ps[hh][0:1, :])
                bc = npool.tile([65, Q], FP32, name=f"bc{hh}", tag=f"bc{hh}")
                nc.gpsimd.partition_broadcast(bc[:], recip[:])
                nc.vector.tensor_mul(xon[2 * m + hh][:], xo_ps[hh][0:65, :],
                                     bc[:])

        phase1_work = [lambda: emit_v(0), lambda: emit_v(1),
                       lambda: emit_v(2), lambda: emit_v(3),
                       lambda: emit_kT(1, 0), lambda: emit_kT(1, 1),
                       lambda: emit_qT(1)]
        pes_live = {}
        xo_tiles = {}
        pending_xo = []
        xo_ready = False

        def drain_xo():
            for m_, kcp_ in pending_xo:
                if m_ not in xo_tiles:
                    xo_tiles[m_] = [
                        xopool.tile([65, Q], FP32, name=f"xo{hh}",
                                    tag=f"xo{hh}") for hh in range(2)]
                emit_xo(m_, kcp_, xo_tiles[m_], pes_live.pop((m_, kcp_)))
                if kcp_ == 7:
                    norm(m_, xo_tiles.pop(m_))
            pending_xo.clear()

        xopool = None
        for gkc in range(32):
            m, kc = gkc // 16, gkc % 16
            kcp, t = kc // 2, kc & 1
            if t == 0:
                pes_live[(m, kcp)] = [
                    pes_pool.tile([128, 2048], FP8E5, name=f"pes{j}",
                                  tag=f"pes{j}") for j in range(2)]
            for j in range(2):
                emit_s_exp(m, kc, j, pes_live[(m, kcp)][j])
            if t == 1:
                pending_xo.append((m, kcp))
            if gkc < 6 and phase1_work:
                phase1_work.pop(0)()
                if gkc == 5:
                    while phase1_work:
                        phase1_work.pop(0)()
                    phase1_ctx.close()
                    xopool = attn_ctx.enter_context(
                        tc.tile_pool(name="xopsum", bufs=1, space="PSUM"))
                    xo_ready = True
            if xo_ready:
                drain_xo()
        drain_xo()

        ot = []
        partialA = []

        def emit_projA():
            for ch in range(2):
                o = spool.tile([128, 1024], FP32, name="s", tag="s")
                ot.append(o)
                for j in range(2):
                    for cc in range(2):
                        nc.tensor.matmul(
                            o[:, j * 512:(j + 1) * 512],
                            lhsT=w_sb["wc"][cc][:, ch * 128:(ch + 1) * 128],
                            rhs=t2T[cc][:, j * 512:(j + 1) * 512],
                            start=(cc == 0), stop=False)
                    for h in range(2):
                        nc.tensor.matmul(
                            o[:, j * 512:(j + 1) * 512],
                            lhsT=wph[h][:, ch * 128:(ch + 1) * 128],
                            rhs=xon[h][:, j * 512:(j + 1) * 512],
                            start=False, stop=False)

        emit_projA()

        for ch in range(2):
            o = ot[ch]
            for j in range(2):
                for h in range(2, 4):
                    nc.tensor.matmul(
                        o[:, j * 512:(j + 1) * 512],
                        lhsT=wph[h][:, ch * 128:(ch + 1) * 128],
                        rhs=xon[h][:, j * 512:(j + 1) * 512],
                        start=False, stop=(h == 3))
            o_sb = osb.tile([128, 1024], BF16, name="o", tag="o")
            nc.vector.tensor_copy(o_sb[:], o[:])
            nc.sync.dma_start(out=out_d[ch * 128:(ch + 1) * 128, :],
                              in_=o_sb[:])

        attn_ctx.close()

    nc.finalize()
    return nc


def _get_nc():
    if "nc" not in _CACHE:
        _CACHE["nc"] = build_nc()
    return _CACHE["nc"]


def make_in_maps(t2_grad, t1, Wq, Wkv, Wproj, bproj):
    bf16 = ml_dtypes.bfloat16
    t2 = np.asarray(t2_grad, dtype=np.float32)
    t1 = np.asarray(t1, dtype=np.float32)
    wq = np.ascontiguousarray(Wq, dtype=np.float32)
    wk = np.ascontiguousarray(Wkv[:, :C]).astype(bf16)
    wv = np.ascontiguousarray(Wkv[:, C:]).astype(bf16)
    wp64 = np.asarray(Wproj, dtype=np.float64)
    wcomb = ((np.eye(C) + np.asarray(Wq, np.float64)) @ wp64).astype(bf16)
    wph = np.zeros((4 * 65, C), dtype=np.float32)
    for h in range(4):
        wph[h * 65 + 1:(h + 1) * 65] = np.asarray(Wproj[h * 64:(h + 1) * 64],
                                                  np.float32)
    wph[0] = np.asarray(bproj, np.float32)
    wph = wph.astype(bf16)
    wq_b = wq.astype(bf16)

    in_maps = []
    for c in range(NCORES):
        b, qh = c // 2, c % 2
        in_maps.append({
            "t1T": np.ascontiguousarray(t1[b].T).astype(bf16),
            "t2T": np.ascontiguousarray(t2[b].T[:, qh * Q:(qh + 1) * Q]).astype(bf16),
            "wq": wq_b, "wk": wk, "wv": wv, "wc": wcomb, "wph": wph,
        })
    return in_maps


def kernel(t2_grad, t1, Wq, Wkv, Wproj, bproj, gamma, _trace=False,
           _use_fp32r=None):
    gamma = np.asarray(gamma)
    if float(np.abs(gamma).max()) != 0.0:
        return _host_reference(t2_grad, t1, Wq, Wkv, Wproj, bproj, gamma)

    nc = _get_nc()
    in_maps = make_in_maps(t2_grad, t1, Wq, Wkv, Wproj, bproj)
    res = run_bass_kernel_spmd(nc, in_maps, list(range(NCORES)), trace=_trace)
    out = np.empty((B, N, C), dtype=np.float32)
    for c in range(NCORES):
        b, qh = c // 2, c % 2
        out[b, qh * Q:(qh + 1) * Q, :] = \
            np.asarray(res.results[c]["out"]).astype(np.float32).T
    if _trace:
        _CACHE["last_result"] = res
    return out


def _host_reference(t2_grad, t1, Wq, Wkv, Wproj, bproj, gamma):
    t2 = np.asarray(t2_grad, dtype=np.float64)
    t1 = np.asarray(t1, dtype=np.float64)
    Wq = np.asarray(Wq, dtype=np.float64)
    Wkv = np.asarray(Wkv, dtype=np.float64)
    Wproj = np.asarray(Wproj, dtype=np.float64)
    bproj = np.asarray(bproj, dtype=np.float64)
    g = float(np.asarray(gamma).reshape(-1)[0])
    q = (t2 @ Wq).reshape(B, N, H, D).transpose(0, 2, 1, 3)
    kv = (t1 @ Wkv).reshape(B, N, 2, H, D).transpose(2, 0, 3, 1, 4)
    k, v = kv[0], kv[1]
    s = np.einsum('bhnd,bhmd->bhnm', q, k) * SCALE
    s = s - s.max(axis=-1, keepdims=True)
    p = np.exp(s)
    p /= p.sum(axis=-1, keepdims=True)
    x = np.einsum('bhnm,bhmd->bhnd', p, v)
    xp = x.transpose(0, 3, 1, 2).reshape(B, D, H * N)
    energy = xp @ xp.transpose(0, 2, 1)
    energy = energy - energy.max(axis=-1, keepdims=True)
    att = np.exp(energy)
    att /= att.sum(axis=-1, keepdims=True)
    lam_out = (att @ xp).reshape(B, D, H, N)
    lam_out = g * lam_out + xp.reshape(B, D, H, N)
    x = lam_out.transpose(0, 2, 3, 1)
    xo = x.transpose(0, 2, 1, 3).reshape(B, N, C) \
        + q.transpose(0, 2, 1, 3).reshape(B, N, C)
    return ((t2 + xo) @ Wproj + bproj).astype(np.float32)
